# revision 1
# baseline (speedup 1.0000x reference)
"""Multi-head attention forward on 8 Trainium2 NeuronCores.

Problem: batch=8, seq=1024, d_model=1024, n_heads=16, d_head=64, fp32 ref.

Sharding: data-parallel over batch - core b computes batch element b end to
end (weights replicated, no collectives).

Per-core layout strategy (nothing ever needs an on-device transpose):
  - x^T (d on partitions) is staged by the host; it serves as
      rhs  for Q^T/K^T = W^T @ x^T   (2 heads packed -> M=128)
      lhsT for V      = x @ W_V      (heads along the free dim)
  - scores^T = K @ Q^T lands with k on partitions, so softmax's exp is one
    ScalarE activation per tile (the 1/sqrt(d) scale and the key-mask fold
    in as activation scale/bias), and the sum over k happens inside the
    P@V matmul via a ones-column appended to V (softmax denominators pop
    out in psum row 64 for free).
  - Z^T = [V|1]^T @ P^T keeps (head, e) on partitions; heads are packed in
    pairs so the output projection contracts with K=128.
  - biases are exact rank-1 (K=1) matmul updates; they are emitted FIRST in
    each accumulation group so the group's psum-reuse (WAR) wait rides on
    them.

Everything is bf16 into the PE with fp32 PSUM accumulation.

This toolchain's walrus encodes at most ONE sync wait per instruction, so
the program is structured to need at most one new foreign-engine tick per
instruction: every load DMA writes a fresh (never reused) tile so it needs
zero waits, a few tiny observer matmuls at the top absorb the DMA-lane
ticks of shared tensors, and phase-internal pipelines are arranged so each
instruction picks up at most one new semaphore.
"""

from contextlib import ExitStack

import numpy as np

import concourse.bass as bass
import concourse.tile as tile
from concourse import mybir
from concourse.bass_utils import run_bass_kernel_spmd

S = 1024  # seq
D = 1024  # d_model
H = 16  # heads
E = 64  # d_head
B = 8  # batch == n_cores
P = 128  # partitions
NS = S // P  # 8 s-tiles
ND = D // P  # 8 d-chunks
NG = H // 2  # 8 head pairs

F32 = mybir.dt.float32
BF16 = mybir.dt.bfloat16
AF = mybir.ActivationFunctionType

MASK_NEG = 60.0  # exp(x - 60) ~ 9e-27: masked keys vanish without inf/nan


def build_program(split_waits=True):
    nc = bass.Bass("TRN2", target_bir_lowering=False, debug=False)

    # all inputs arrive pre-packed by the host into their exact SBUF layouts
    xt_d = nc.dram_tensor("xt", [P, ND, S], BF16, kind="ExternalInput").ap()
    wq_d = nc.dram_tensor("wq", [P, NG, ND, P], BF16, kind="ExternalInput").ap()
    wk_d = nc.dram_tensor("wk", [P, NG, ND, P], BF16, kind="ExternalInput").ap()
    wv_d = nc.dram_tensor("wv", [P, ND, H * E], BF16, kind="ExternalInput").ap()
    wo_d = nc.dram_tensor("wo", [P, NG, D], BF16, kind="ExternalInput").ap()
    # packed small constants: [b_Q | b_K | b_V | b_O], each flattened to 1024
    cpk_d = nc.dram_tensor("cpk", [1, 4 * 1024], BF16, kind="ExternalInput").ap()
    mb_d = nc.dram_tensor("mb", [P, NS], F32, kind="ExternalInput").ap()
    out_d = nc.dram_tensor("out", [S, D], F32, kind="ExternalOutput").ap()

    with tile.TileContext(nc) as tc, ExitStack() as ctx:
        g1 = ctx.enter_context(tc.tile_pool(name="g1", bufs=1))

        ones_row = g1.tile([1, S], BF16, tag="ones_row")
        nc.vector.memset(ones_row, 1.0)
        ones_col = g1.tile([1, P], BF16, tag="ones_col")
        nc.vector.memset(ones_col, 1.0)
        mb_sb = g1.tile([P, NS], F32, tag="mb")
        nc.sync.dma_start(out=mb_sb, in_=mb_d)
        cpk = g1.tile([1, 4 * 1024], BF16, tag="cpk")
        nc.sync.dma_start(out=cpk, in_=cpk_d)
        bq_sb = cpk[:, 0 : H * E]
        bk_sb = cpk[:, H * E : 2 * H * E]
        bv_sb = cpk[:, 2 * H * E : 3 * H * E]
        bo_sb = cpk[:, 3 * H * E : 4 * H * E]

        # xT: [d%128, d-chunk, s] - one DMA, one semaphore lane
        xT = g1.tile([P, ND, S], BF16, tag="xT")
        nc.sync.dma_start(out=xT, in_=xt_d)
        # wo: [(h%2)*64+e, pair g, d] - one DMA
        wo_sb = g1.tile([P, NG, D], BF16, tag="wo_sb")
        nc.sync.dma_start(out=wo_sb, in_=wo_d)

        # persistent activations
        qT = g1.tile([P, NG, S], BF16, tag="qT")
        kT = g1.tile([P, NG, S], BF16, tag="kT")
        vb = g1.tile([P, NS, H, E + 1], BF16, tag="vb")
        nc.vector.memset(vb, 1.0)  # pre-sets the softmax-sum ones columns
        zT = g1.tile([P, NG, S], BF16, tag="zT")

        # observer ldweights: absorb one new semaphore tick each on PE, so
        # later consumers of these tensors carry at most one wait themselves.
        nc.tensor.ldweights(ones_col)  # DVE tick (memsets)
        nc.tensor.ldweights(cpk[:, 0:P])  # cpk DMA lane
        nc.tensor.ldweights(xT[:, 0, 0:8])  # xT DMA lane
        nc.tensor.ldweights(wo_sb[:, 0, 0:8])  # wo DMA lane
        act_scrap = g1.tile([P, 1], F32, tag="act_scrap")
        nc.scalar.activation(  # mb DMA lane, observed by ScalarE
            out=act_scrap, in_=mb_sb[:, 0:1], func=AF.Copy
        )

        _projections(nc, tc, xT, wq_d, wk_d, wv_d, qT, kT, vb,
                     bq_sb, bk_sb, bv_sb, ones_row, ones_col)
        _attention(nc, tc, qT, kT, vb, zT, mb_sb, ones_row, ones_col)
        _out_proj(nc, tc, zT, wo_sb, bo_sb, ones_col, out_d)

    if split_waits:
        _split_multi_waits(nc)
    return nc


def _split_multi_waits(nc):
    """This walrus build encodes at most ONE sync wait per instruction.
    Tile emits more. Hoist excess waits onto same-engine EventSemaphore
    instructions inserted immediately before the offender - engines and
    DGE sequencers execute their streams in order, so this preserves
    semantics exactly."""
    n = 0
    for fn in nc.m.functions:
        for bb in fn.blocks:
            out = []
            for inst in bb.instructions:
                si = getattr(inst, "sync_info", None)
                waits = list(si.on_wait) if si is not None and si.on_wait else []
                if len(waits) > 1:
                    for w in waits[:-1]:
                        n += 1
                        out.append(
                            mybir.InstEventSemaphore(
                                name=f"evw-{n}",
                                engine=inst.engine,
                                sync_info=mybir.SyncInfo(
                                    on_wait=[w], on_update=[]
                                ),
                            )
                        )
                    si.on_wait = [waits[-1]]
                out.append(inst)
            bb.instructions[:] = out


def _projections(nc, tc, xT, wq_d, wk_d, wv_d, qT, kT, vb,
                 bq_sb, bk_sb, bv_sb, ones_row, ones_col):
    with (
        tc.tile_pool(name="wqk", bufs=1) as wqkp,
        tc.tile_pool(name="wvp", bufs=1) as wvp,
        tc.tile_pool(name="qp", bufs=4, space="PSUM") as qpp,
        tc.tile_pool(name="vp", bufs=2, space="PSUM") as vpp,
    ):
        # resident weights, each loaded write-once
        # wq/wk: [d%128, pair g, d-chunk, (2 heads x 64)]
        wq_sb = wqkp.tile([P, NG, ND, P], BF16, tag="wq_sb")
        wk_sb = wqkp.tile([P, NG, ND, P], BF16, tag="wk_sb")
        nc.sync.dma_start(out=wq_sb, in_=wq_d)
        nc.sync.dma_start(out=wk_sb, in_=wk_d)
        # wv: [d%128, d-chunk, (16 heads x 64)]
        wv_sb = wvp.tile([P, ND, H * E], BF16, tag="wv_sb")
        nc.sync.dma_start(out=wv_sb, in_=wv_d)

        # Q^T / K^T per head pair; bias rank-1 matmul FIRST in each group
        for dst, w_sb, b_sb in ((qT, wq_sb, bq_sb), (kT, wk_sb, bk_sb)):
            for g in range(NG):
                qps = [qpp.tile([P, S // 2], F32, tag="qp", name=f"qp{g}{i}") for i in range(2)]
                for qh in range(2):
                    nc.tensor.matmul(
                        out=qps[qh],
                        lhsT=b_sb[:, g * P : (g + 1) * P],
                        rhs=ones_row[:, qh * 512 : qh * 512 + 512],
                        start=True,
                        stop=False,
                    )
                for c in range(ND):
                    for qh in range(2):  # same lhsT back-to-back
                        nc.tensor.matmul(
                            out=qps[qh],
                            lhsT=w_sb[:, g, c, :],
                            rhs=xT[:, c, qh * 512 : (qh + 1) * 512],
                            start=False,
                            stop=(c == ND - 1),
                        )
                for qh in range(2):
                    nc.vector.tensor_copy(
                        out=dst[:, g, qh * 512 : (qh + 1) * 512], in_=qps[qh]
                    )

        # V = x @ W_V + b_V in two 8-head halves, stored bf16 into vb
        for st in range(NS):
            vps = [vpp.tile([P, 512], F32, tag="vp", name=f"vp{st}{i}") for i in range(2)]
            for hh in range(2):  # same lhsT (ones) back-to-back
                nc.tensor.matmul(
                    out=vps[hh],
                    lhsT=ones_col,
                    rhs=bv_sb[:, hh * 512 : (hh + 1) * 512],
                    start=True,
                    stop=False,
                )
            for c in range(ND):
                for hh in range(2):  # same lhsT (xT chunk) back-to-back
                    nc.tensor.matmul(
                        out=vps[hh],
                        lhsT=xT[:, c, st * P : (st + 1) * P],
                        rhs=wv_sb[:, c, hh * 512 : (hh + 1) * 512],
                        start=False,
                        stop=(c == ND - 1),
                    )
            for hh in range(2):
                nc.vector.tensor_copy(
                    out=vb[:, st, hh * 8 : (hh + 1) * 8, 0:E],
                    in_=vps[hh].rearrange("p (h e) -> p h e", h=8),
                )



def _attention(nc, tc, qT, kT, vb, zT, mb_sb, ones_row, ones_col):
    with (
        tc.tile_pool(name="pt", bufs=2) as ptp,
        tc.tile_pool(name="rcp", bufs=1) as rcp,
        tc.tile_pool(name="sm", bufs=4) as smp,
        tc.tile_pool(name="st", bufs=2, space="PSUM") as stp,
        tc.tile_pool(name="zps", bufs=4, space="PSUM") as zpsp,
    ):
        # write-once strip of softmax denominator reciprocals (bf16)
        rc_all = rcp.tile([1, H, 2, 512], BF16, tag="rc_all")
        for h in range(H):
            g, half = h // 2, h % 2
            pt = ptp.tile([P, NS, S], BF16, tag="pt", name=f"pt{h}")
            for kt in range(NS):
                st_ps = stp.tile([P, S], F32, tag="st", name=f"st{h}{kt}")
                for qh in range(2):
                    nc.tensor.matmul(
                        out=st_ps[:, qh * 512 : (qh + 1) * 512],
                        lhsT=kT[half * E : (half + 1) * E, g,
                                kt * P : (kt + 1) * P],
                        rhs=qT[half * E : (half + 1) * E, g,
                               qh * 512 : (qh + 1) * 512],
                        start=True,
                        stop=True,
                    )
                nc.scalar.activation(
                    out=pt[:, kt, :],
                    in_=st_ps,
                    func=AF.Exp,
                    bias=mb_sb[:, kt : kt + 1],
                    scale=0.125,
                )
            if h == 0:
                nc.tensor.ldweights(vb[:, NS - 1, H - 1, :])
            zps = [zpsp.tile([E + 1, 512], F32, tag="zp", name=f"zp{h}{i}") for i in range(2)]
            for kt in range(NS):
                for qh in range(2):
                    nc.tensor.matmul(
                        out=zps[qh],
                        lhsT=vb[:, kt, h, :],
                        rhs=pt[:, kt, qh * 512 : (qh + 1) * 512],
                        start=(kt == 0),
                        stop=(kt == NS - 1),
                    )
            for qh in range(2):
                zp = zps[qh]
                rc = rc_all[:, h, qh, :]
                with nc.allow_low_precision(reason="bf16 softmax denom"):
                    nc.vector.reciprocal(out=rc, in_=zp[E : E + 1, :])
                bc = smp.tile([E, 512], BF16, tag="bc", name=f"bc{h}{qh}")
                nc.sync.dma_start(
                    out=bc, in_=rc.unsqueeze(1).broadcast_to((1, E, 512))
                )
                nc.vector.tensor_mul(
                    zT[half * E : (half + 1) * E, g, qh * 512 : (qh + 1) * 512],
                    zp[0:E, :],
                    bc,
                )


def _out_proj(nc, tc, zT, wo_sb, bo_sb, ones_col, out_d):
    with (
        tc.tile_pool(name="ob", bufs=1) as obp,
        tc.tile_pool(name="op", bufs=4, space="PSUM") as opp,
    ):
        for st in range(NS):
            ob = obp.tile([P, D], F32, tag=f"ob{st}")  # write-once per s-tile
            ops = [opp.tile([P, 512], F32, tag="op", name=f"op{st}{i}") for i in range(2)]
            for dh in range(2):  # same lhsT (ones) back-to-back
                nc.tensor.matmul(
                    out=ops[dh],
                    lhsT=ones_col,
                    rhs=bo_sb[:, dh * 512 : (dh + 1) * 512],
                    start=True,
                    stop=False,
                )
            for g in range(NG):
                for dh in range(2):  # same lhsT (zT slice) back-to-back
                    nc.tensor.matmul(
                        out=ops[dh],
                        lhsT=zT[:, g, st * P : (st + 1) * P],
                        rhs=wo_sb[:, g, dh * 512 : (dh + 1) * 512],
                        start=False,
                        stop=(g == NG - 1),
                    )
            for dh in range(2):
                nc.vector.tensor_copy(
                    out=ob[:, dh * 512 : (dh + 1) * 512], in_=ops[dh]
                )
            nc.sync.dma_start(out=out_d[st * P : (st + 1) * P, :], in_=ob)


_NC_CACHE = None


def _get_nc():
    global _NC_CACHE
    if _NC_CACHE is None:
        _NC_CACHE = build_program()
    return _NC_CACHE


def _make_in_maps(inputs):
    import ml_dtypes

    bf16 = ml_dtypes.bfloat16
    x = np.asarray(inputs["x"], np.float32)
    mask = np.asarray(inputs["key_attention_mask"])
    wq = np.asarray(inputs["W_Q"], np.float32).astype(bf16)
    wk = np.asarray(inputs["W_K"], np.float32).astype(bf16)
    wv = np.asarray(inputs["W_V"], np.float32).astype(bf16)
    wo = np.asarray(inputs["W_O"], np.float32).astype(bf16)

    def pack_qk(w):  # (H, D, E) -> [p, g, c, (h2 e)]
        return np.ascontiguousarray(
            w.reshape(NG, 2, ND, P, E).transpose(3, 0, 2, 1, 4).reshape(P, NG, ND, P)
        )

    cpk = np.concatenate([
        np.asarray(inputs["b_Q"], np.float32).reshape(-1),
        np.asarray(inputs["b_K"], np.float32).reshape(-1),
        np.asarray(inputs["b_V"], np.float32).reshape(-1),
        np.asarray(inputs["b_O"], np.float32).reshape(-1),
    ]).reshape(1, 4 * 1024)
    shared = {
        "wq": pack_qk(wq),
        "wk": pack_qk(wk),
        # (H, D, E) -> [p, c, (h e)]
        "wv": np.ascontiguousarray(
            wv.reshape(H, ND, P, E).transpose(2, 1, 0, 3).reshape(P, ND, H * E)
        ),
        # (H, E, D) -> [(h2 e), g, d]
        "wo": np.ascontiguousarray(
            wo.reshape(NG, 2, E, D).transpose(1, 2, 0, 3).reshape(P, NG, D)
        ),
        "cpk": cpk.astype(bf16),
    }
    in_maps = []
    for b in range(B):
        m = dict(shared)
        xt = x[b].T.astype(bf16)  # (D, S) -> [p, c, s]
        m["xt"] = np.ascontiguousarray(
            xt.reshape(ND, P, S).transpose(1, 0, 2)
        )
        mb = ((mask[b] != 0).astype(np.float32) - 1.0) * MASK_NEG
        m["mb"] = np.ascontiguousarray(mb.reshape(NS, P).T)
        in_maps.append(m)
    return in_maps


def run(inputs, trace=False):
    nc = _get_nc()
    res = run_bass_kernel_spmd(nc, _make_in_maps(inputs), list(range(B)),
                               trace=trace)
    out = np.stack([res.results[b]["out"] for b in range(B)], axis=0)
    return out, res


def kernel(**inputs) -> np.ndarray:
    out, _ = run(inputs, trace=False)
    return out

